# revision 20
# baseline (speedup 1.0000x reference)
"""Multi-head attention (b=4, n=4096, d_model=768, 16 heads x 128) on 8 TRN2
NeuronCores.

Sharding: core c handles batch c//2, head-group c%2 (8 heads = 1024 channels).
Host transposes q/k/v per batch to (768, n) and slices weights per head-group;
each core computes its partial output projection (n, 768); host sums the two
head-group partials per batch and adds the output bias.

Per-core dataflow (layouts chosen so no probability transpose is needed):
  phase 1a: qhT/khT = per-head projections producing (head_dim, n) via
           out = lhsT.T @ rhs with lhsT=W chunk, rhs=xT chunk; q/k bias added
           on ScalarE (Identity activation with per-partition bias AP).
  phase 1b: vh in natural (n, ch) layout, NO bias (v-bias folded in post-PV:
           out = pv/denom + bv since sum(probs)=denom).
  phase 2: flat software pipeline over (head, qt) steps that crosses head
           boundaries: each step emits score matmuls + exp for (h, qt)
           interleaved with the PV chains of the previous step's tile, so the
           PE never drains at head starts (scores alone are exp-throughput
           gated; PV work covers the drain).  scoresT (keys, q) =
           khT_chunk.T @ qhT -> probsT via exp split across ScalarE (exact
           Exp) and VectorE (Schraudolph fast-exp) in a 5:4 ratio.  PV per
           q-block: out(q,129) = probsT_chunk.T @ [vh | 1] accumulating the
           softmax denominator in column 128; normalize on VectorE fused with
           the v-bias add, DMA-xbar transpose into attnT (ch, n).
  phase 3: out(n,768) = attnT_chunk.T @ Wo chunks, fp32 out.  Interleaved
           into the last head's pipeline steps (nb-groups emitted two steps
           behind the PV that completes them), with a smaller score PSUM pool
           for that head so PSUM fits scores+PV+out chains simultaneously.
"""

import numpy as np
import ml_dtypes
from contextlib import ExitStack

import concourse.bass as bass
import concourse.mybir as mybir
import concourse.tile as tile
from concourse import bacc
from concourse.bass_utils import run_bass_kernel_spmd

B = 4
N_CTX = 4096
DM = 768
CH = 1024          # channels per core (8 heads x 128)
HD = 128
NH = 8             # heads per core
KC = DM // 128     # 6 contraction chunks for projections
NT = 512           # n-tile width
SCALE = HD ** -0.5
LOG2E = 1.4426950408889634
C_ZM = 7.219274    # zero-mean Schraudolph constant (2^7 mantissa domain)

f32 = mybir.dt.float32
bf16 = mybir.dt.bfloat16
i16 = mybir.dt.int16
fp8 = mybir.dt.float8e4
AF = mybir.ActivationFunctionType
ALU = mybir.AluOpType
PM = mybir.MatmulPerfMode
NTP = 1024         # phase-1 n-tile width (wide moving to amortize LDWEIGHTS)

_CACHE = {}


def _install_profhook():
    import contextlib, ctypes, sys, types

    if "antenv.axon_hooks" in sys.modules:
        return
    so = "/opt/axon/libaxon_pjrt.so"
    try:
        lib = ctypes.CDLL(so)
    except OSError:
        return
    if not hasattr(lib, "axon_start_nrt_profile"):
        return
    lib.axon_start_nrt_profile.argtypes = [ctypes.POINTER(ctypes.c_int64), ctypes.c_size_t]
    lib.axon_start_nrt_profile.restype = ctypes.c_int64
    lib.axon_stop_nrt_profile.argtypes = [ctypes.c_char_p]
    lib.axon_stop_nrt_profile.restype = ctypes.c_int64

    @contextlib.contextmanager
    def _hook(output_dir, device_ids):
        import jax
        jax.devices()
        if device_ids:
            ids = (ctypes.c_int64 * len(device_ids))(*device_ids)
            rc = lib.axon_start_nrt_profile(ids, len(device_ids))
        else:
            rc = lib.axon_start_nrt_profile(None, 0)
        if rc != 0:
            raise RuntimeError(f"axon_start_nrt_profile rc={rc}")
        try:
            yield
        finally:
            nf = lib.axon_stop_nrt_profile(str(output_dir).encode())
            print(f"profile: {nf} ntff file(s) in {output_dir}", file=sys.stderr)

    mod = types.ModuleType("antenv.axon_hooks")
    mod.get_axon_ntff_profile_hook = lambda: _hook
    mod.set_axon_ntff_profile_hook = lambda h: None
    sys.modules["antenv.axon_hooks"] = mod

    import concourse.bass_utils as bu
    bu.upload_artifacts = lambda tmpdir: "local://" + str(tmpdir)


def _build(n=N_CTX):
    nts = n // NT          # n tiles (8)
    nbs = n // 128         # n blocks (32)
    nc = bacc.Bacc(None, target_bir_lowering=False, debug=False, num_devices=8)

    qT = nc.declare_dram_parameter("qT", [DM, n], fp8, isOutput=False)
    kT = nc.declare_dram_parameter("kT", [DM, n], fp8, isOutput=False)
    vT = nc.declare_dram_parameter("vT", [DM, n], bf16, isOutput=False)
    wq = nc.declare_dram_parameter("wq", [DM, CH], fp8, isOutput=False)
    wk = nc.declare_dram_parameter("wk", [DM, CH], fp8, isOutput=False)
    wv = nc.declare_dram_parameter("wv", [DM, CH], bf16, isOutput=False)
    wo = nc.declare_dram_parameter("wo", [CH, DM], bf16, isOutput=False)
    bqk = nc.declare_dram_parameter("bqk", [128, 2 * NH], f32, isOutput=False)
    bvb = nc.declare_dram_parameter("bvb", [128, CH], f32, isOutput=False)
    out = nc.declare_dram_parameter("out", [n, DM], f32, isOutput=True)

    with tile.TileContext(nc) as tc, ExitStack() as ctx:
        dram = ctx.enter_context(tc.tile_pool(name="dram", bufs=1, space="DRAM"))
        qh_s = dram.tile([NH, 128, n], bf16)
        kh_s = dram.tile([NH, 128, n], bf16)
        vh_s = dram.tile([nbs, 128, NH, HD], bf16)

        singles = ctx.enter_context(tc.tile_pool(name="singles", bufs=1))

        bqk_t = singles.tile([128, 2 * NH], f32)
        nc.sync.dma_start(out=bqk_t[:, :], in_=bqk[:, :])
        bvb_t = singles.tile([128, CH], f32)
        nc.sync.dma_start(out=bvb_t[:, :], in_=bvb[:, :])
        # warm the ScalarE exp table set while phase 1 hasn't touched ACT yet
        # (the first real Exp otherwise pays the ~2.7us ACT_TABLE_LOAD right
        # at the phase-1 -> phase-2 transition, stalling the score pipeline)
        scratch = singles.tile([128, 1], f32)
        nc.scalar.activation(scratch[:, :], bqk_t[:, 0:1], AF.Exp)

        # phase-2 pools that phase 1 needs (head-0 score overlap): qh/kh tiles
        # and the probs pool must exist before the last v-projection tile
        GG = 16  # key-chunks per buffered probs tile (2 tiles per qt)
        EG = 3   # key-chunks per exp op (3 psum banks per sc tile)
        sch_s1 = SCALE * LOG2E * 128.0
        sch_s2 = 127.0 * 128.0 - C_ZM
        qk2 = ctx.enter_context(tc.tile_pool(name="qk2", bufs=2))
        probs = ctx.enter_context(tc.tile_pool(name="probs", bufs=4))
        qk_ts = {}
        pq_by = {}    # (h, qt) -> list of probs tiles

        def load_qk(h):
            qh_t = qk2.tile([128, n], bf16, tag="qh", name=f"qh{h}")
            kh_t = qk2.tile([128, n], bf16, tag="kh", name=f"kh{h}")
            for chk in range(n // NTP):
                cs = slice(chk * NTP, (chk + 1) * NTP)
                nc.sync.dma_start(out=qh_t[:, cs], in_=qh_s[h, :, cs])
                nc.sync.dma_start(out=kh_t[:, cs], in_=kh_s[h, :, cs])
            return qh_t, kh_t

        # ---------------- phase 1a: q/k projections ----------------
        p1 = ExitStack()
        wpool = p1.enter_context(tc.tile_pool(name="w1", bufs=1))
        stream = p1.enter_context(tc.tile_pool(name="stream", bufs=3))
        stage1 = p1.enter_context(tc.tile_pool(name="stage1", bufs=4))
        pp = p1.enter_context(tc.tile_pool(name="pp", bufs=4, space="PSUM"))

        wq_t = wpool.tile([128, KC, CH], fp8, tag="wq")
        wk_t = wpool.tile([128, KC, CH], fp8, tag="wk")
        wv_t = wpool.tile([128, KC, CH], bf16, tag="wv")
        for w_t, w in ((wq_t, wq), (wk_t, wk), (wv_t, wv)):
            nc.sync.dma_start(
                out=w_t[:, :, :], in_=w[:].rearrange("(c p) m -> p c m", p=128)
            )

        ntsp = n // NTP
        for nt in range(ntsp):
            xs = []
            for name, x in (("q", qT), ("k", kT)):
                x_t = stream.tile([128, KC, NTP], fp8, tag=f"s{name}")
                # per-chunk-pair DMAs: the first DR matmul needs only pair 0,
                # so projections start before the whole tile lands
                for cp_ in range(KC // 2):
                    nc.sync.dma_start(
                        out=x_t[:, 2 * cp_ : 2 * cp_ + 2, :],
                        in_=x[:].rearrange("(c p) m -> p c m", p=128)[
                            :, 2 * cp_ : 2 * cp_ + 2, nt * NTP : (nt + 1) * NTP
                        ],
                    )
                xs.append(x_t)
            xv_t = stream.tile([128, KC, NTP], bf16, tag="sv", bufs=2)
            nc.sync.dma_start(
                out=xv_t[:, :, :],
                in_=vT[:].rearrange("(c p) m -> p c m", p=128)[
                    :, :, nt * NTP : (nt + 1) * NTP
                ],
            )
            for ti, (x_t, sc) in enumerate(((xs[0], qh_s), (xs[1], kh_s))):
                w_t = (wq_t, wk_t)[ti]
                for h in range(NH):
                    ps = pp.tile([128, NTP], f32, tag="proj")
                    # fp8 DoubleRow: contract chunk pair (2c, 2c+1); fp8
                    # moving operand caps at 1024 elems so go per 512-half
                    for half in range(NTP // 512):
                        hs = slice(half * 512, (half + 1) * 512)
                        for c in range(KC // 2):
                            nc.tensor.matmul(
                                ps[:, hs],
                                w_t[:, 2 * c : 2 * c + 2, h * HD : (h + 1) * HD],
                                x_t[:, 2 * c : 2 * c + 2, hs],
                                start=(c == 0),
                                stop=(c == KC // 2 - 1),
                                perf_mode=PM.DoubleRow,
                                skip_group_check=True,
                            )
                    st = stage1.tile([128, NTP], bf16, tag="qkst")
                    # bias add on ScalarE (idle during phase 1)
                    nc.scalar.activation(
                        st[:, :], ps[:, :], AF.Identity,
                        bias=bqk_t[:, ti * NH + h : ti * NH + h + 1],
                    )
                    nc.sync.dma_start(
                        out=sc[h, :, nt * NTP : (nt + 1) * NTP], in_=st[:, :]
                    )
            # v projection for this n-tile (no bias; folded post-PV)
            def emit_vproj(nt_, nbq, xv):
                ps = pp.tile([128, CH], f32, tag="proj", name="psv")
                for ct in range(CH // 512):
                    cs = slice(ct * 512, (ct + 1) * 512)
                    for c in range(KC):
                        nc.tensor.matmul(
                            ps[:, cs],
                            xv[:, c, nbq * 128 : (nbq + 1) * 128],
                            wv_t[:, c, cs],
                            start=(c == 0),
                            stop=(c == KC - 1),
                            skip_group_check=True,
                        )
                st = stage1.tile([128, CH], bf16, tag="vst")
                nc.vector.tensor_copy(st[:, :], ps[:, :])
                nc.sync.dma_start(
                    out=vh_s[nt_ * (NTP // 128) + nbq, :, :, :], in_=st[:, :],
                )

            if nt < ntsp - 1:
                for nbq in range(NTP // 128):
                    emit_vproj(nt, nbq, xv_t)
            else:
                # last n-tile: interleave head-0 qt-0 scores with the
                # v-projection so the transition into phase 2 never drains the
                # PE (score groups alone are exp-throughput gated). Score psums
                # borrow the phase-1 "proj" pool slots (eg=2 -> same 4KB size).
                qk_ts[0] = load_qk(0)
                pq00 = []
                for gq in range(2):
                    pq0 = probs.tile([128, GG, NT], bf16, tag="pq",
                                     name=f"pq0_0_{gq}")
                    pq00.append(pq0)
                sg00 = [(pq00[g // 8], (g // 8) * GG, (g % 8) * 2)
                        for g in range(16)]
                gi = 0
                sched = {2: 2, 3: 3, 4: 3, 5: 3, 6: 3, 7: 2}

                def emit_sg00(task):
                    pq0, kb0, off = task
                    sc_t = pp.tile([128, 2, NT], f32, tag="proj", name="sc00")
                    for i in range(2):
                        kb = kb0 + off + i
                        nc.tensor.matmul(
                            sc_t[:, i, :],
                            qk_ts[0][1][:, kb * 128 : (kb + 1) * 128],
                            qk_ts[0][0][:, 0:NT],
                            start=True, stop=True,
                        )
                    # 2 ScalarE : 1 VectorE here — the DVE carries the
                    # v-projection psum drains in this window
                    if gi % 3 < 2:
                        nc.scalar.activation(
                            pq0[:, off : off + 2, :], sc_t[:, :, :],
                            AF.Exp, scale=SCALE,
                        )
                    else:
                        nc.vector.tensor_scalar(
                            pq0[:, off : off + 2, :].bitcast(i16),
                            sc_t[:, :, :],
                            sch_s1, sch_s2, ALU.mult, ALU.add,
                        )

                for nbq in range(NTP // 128):
                    emit_vproj(nt, nbq, xv_t)
                    for _ in range(sched.get(nbq, 0)):
                        emit_sg00(sg00[gi]); gi += 1
                while gi < 16:
                    emit_sg00(sg00[gi]); gi += 1
                pq_by[(0, 0)] = pq00

        p1.close()

        # ---------------- phase 2+3: flat pipelined attention ----------------
        atile = ctx.enter_context(tc.tile_pool(name="atile", bufs=NH))
        vh2 = ctx.enter_context(tc.tile_pool(name="vh2", bufs=2))
        stage2 = ctx.enter_context(tc.tile_pool(name="stage2", bufs=4))
        small = ctx.enter_context(tc.tile_pool(name="small", bufs=4))
        stage3 = ctx.enter_context(tc.tile_pool(name="stage3", bufs=2))
        wpool3 = ctx.enter_context(tc.tile_pool(name="w3", bufs=1))
        pvp = ctx.enter_context(tc.tile_pool(name="pvp", bufs=2, space="PSUM"))

        # output-projection weights: load early, plenty of DMA slack
        wo_t = wpool3.tile([128, NH, DM], bf16, tag="wo")
        nc.sync.dma_start(
            out=wo_t[:, :, :], in_=wo[:].rearrange("(c p) m -> p c m", p=128)
        )

        # scores psum: big pool for heads 0..NH-2, small for the last head so
        # the interleaved phase-3 chains get PSUM banks
        pA = ExitStack()
        scpA = pA.enter_context(tc.tile_pool(name="scpA", bufs=2, space="PSUM"))
        pB1 = pB2 = pB3 = None
        scpB = None
        opp = None
        opp2 = None

        def load_vh(h):
            vh_t = vh2.tile([128, nbs, HD + 1], bf16, tag="vh", name=f"vh{h}")
            for chk in range(4):
                bs = slice(chk * (nbs // 4), (chk + 1) * (nbs // 4))
                nc.sync.dma_start(
                    out=vh_t[:, bs, 0:HD],
                    in_=vh_s[bs, :, h, :].rearrange("c p j -> p c j"),
                )
            nc.vector.memset(vh_t[:, :, HD], 1.0)
            return vh_t

        vh_ts = {0: load_vh(0)}
        at_ts = []
        expctr = 1    # global exp-group counter; strict scalar/vector
        # alternation (a 5:4 pattern stalls the score psum rotation at its
        # wrap where two consecutive groups land on ScalarE). Starts at 1:
        # measured marginally better rel-err than the even-parity start, same
        # speed.

        def emit_sg(task):
            nonlocal expctr
            scp, eg, pq, kb0, off, gsz, h, qt = task
            kh_t = qk_ts[h][1]
            qh_t = qk_ts[h][0]
            sc_t = scp.tile([128, eg, NT], f32, tag="sc")
            for i in range(gsz):
                kb = kb0 + off + i
                nc.tensor.matmul(
                    sc_t[:, i, :],
                    kh_t[:, kb * 128 : (kb + 1) * 128],
                    qh_t[:, qt * NT : (qt + 1) * NT],
                    start=True,
                    stop=True,
                )
            if expctr % 2 == 0:
                nc.scalar.activation(
                    pq[:, off : off + gsz, :], sc_t[:, 0:gsz, :],
                    AF.Exp, scale=SCALE,
                )
            else:
                nc.vector.tensor_scalar(
                    pq[:, off : off + gsz, :].bitcast(i16),
                    sc_t[:, 0:gsz, :],
                    sch_s1, sch_s2, ALU.mult, ALU.add,
                )
            expctr += 1

        def emit_pv(h, qt, qb):
            plist = pq_by[(h, qt)]
            vh_t = vh_ts[h]
            at_t = at_ts[h]
            pv = pvp.tile([128, HD + 1], f32, tag="pv")
            # high priority: when both a PV matmul and a score matmul are
            # ready, prefer PV — score groups are gated by the sc-psum
            # rotation (exp drain) and stall the in-order PE queue if placed
            # too early (the cost model underestimates DR matmul time, so the
            # scheduler is systematically optimistic about exp readiness)
            with tc.high_priority(offset=200):
                for kb in range(nbs):
                    nc.tensor.matmul(
                        pv[:, :],
                        plist[kb // GG][:, kb % GG, qb * 128 : (qb + 1) * 128],
                        vh_t[:, kb, :],
                        start=(kb == 0),
                        stop=(kb == nbs - 1),
                        skip_group_check=True,
                    )
            rec = small.tile([128, 1], f32, tag="rec")
            nc.vector.reciprocal(rec[:, :], pv[:, HD : HD + 1])
            st = stage2.tile([128, HD], bf16, tag="nst")
            # st = pv/denom + bv  (one fused DVE op)
            nc.vector.scalar_tensor_tensor(
                st[:, :], pv[:, 0:HD], rec[:, :],
                bvb_t[:, h * HD : (h + 1) * HD],
                ALU.mult, ALU.add,
            )
            qb_g = qt * 4 + qb
            # transpose via DMA xbar (frees TensorE + avoids queue stall)
            nc.sync.dma_start_transpose(
                out=at_t[:, qb_g * 128 : (qb_g + 1) * 128], in_=st[:, :]
            )

        def emit_p3(nb, pool=None):
            po = (pool or opp).tile([128, DM], f32, tag="po")
            for hh in range(NH):
                lhsT = at_ts[hh][:, nb * 128 : (nb + 1) * 128]
                nc.tensor.matmul(
                    po[:, 0:512], lhsT, wo_t[:, hh, 0:512],
                    start=(hh == 0), stop=(hh == NH - 1), skip_group_check=True,
                )
                nc.tensor.matmul(
                    po[:, 512:DM], lhsT, wo_t[:, hh, 512:DM],
                    start=(hh == 0), stop=(hh == NH - 1), skip_group_check=True,
                )
            so = stage3.tile([128, DM], f32, tag="ost")
            # drain on ScalarE: the last head's DVE is busy with exp + PV
            # normalization; ACT has slack there
            nc.scalar.activation(so[:, :], po[:, :], AF.Identity)
            nc.sync.dma_start(out=out[nb * 128 : (nb + 1) * 128, :], in_=so[:, :])

        # step list: (h, qt) with qt < nts emits scores for (h, qt); every step
        # emits PV for the previous (h, qt) pair; last head gets a sentinel
        # step and interleaved phase-3 groups (two steps behind).
        steps = [(h, qt) for h in range(NH) for qt in range(nts)]
        steps.append((NH - 1, nts))  # sentinel: final PV + phase-3 tail

        for h, qt in steps:
            last_head = h == NH - 1
            if qt == 0:
                at_t = atile.tile([128, n], bf16, tag="at", name=f"at{h}")
                at_ts.append(at_t)
                if h + 1 < NH:
                    qk_ts[h + 1] = load_qk(h + 1)
                if h == NH - 1:
                    # switch to the narrower scores pool (EG=2, still double
                    # buffered) + open phase-3 psum
                    pA.close()
                    pB2 = ExitStack()
                    opp = pB2.enter_context(
                        tc.tile_pool(name="opp", bufs=1, space="PSUM"))
                    pB1 = ExitStack()
                    scpB = pB1.enter_context(
                        tc.tile_pool(name="scpB", bufs=2, space="PSUM"))
            if qt == 1 and h + 1 < NH:
                vh_ts[h + 1] = load_vh(h + 1)

            scp = scpB if last_head else scpA
            eg = 2 if last_head else EG

            # score-group tasks for this (h, qt); (0,0) was emitted inside
            # phase 1, interleaved with the last v-projection tile
            sg = []
            if qt < nts and not (h == 0 and qt == 0):
                pq_list = []
                for gq in range(nbs // GG):
                    kb0 = gq * GG
                    pq = probs.tile([128, GG, NT], bf16, tag="pq",
                                    name=f"pq{h}_{qt}_{gq}")
                    pq_list.append(pq)
                    off = 0
                    while off < GG:
                        gsz = min(eg, GG - off)
                        sg.append((scp, eg, pq, kb0, off, gsz, h, qt))
                        off += gsz
                pq_by[(h, qt)] = pq_list

            # PV chains for the previous step's scores
            if qt > 0:
                pv_h, pv_qt = h, qt - 1
            elif h > 0:
                pv_h, pv_qt = h - 1, nts - 1
            else:
                pv_h = None
            npv = 4 if pv_h is not None else 0

            # phase-3 groups: two steps behind the PV completion on last head
            p3_nbs = []
            if last_head and qt >= 2:
                blk = qt - 2
                p3_nbs = list(range(blk * 4, blk * 4 + 4))
            if last_head and qt == nts:
                p3_nbs += list(range((nts - 1) * 4, (nts - 1) * 4 + 4))
                # scores are done: free the score psum and open a second
                # phase-3 psum so the tail chains double-buffer their drains
                pB1.close()
                pB3 = ExitStack()
                opp2 = pB3.enter_context(
                    tc.tile_pool(name="opp2", bufs=1, space="PSUM"))

            # interleave: 2 score groups per PV chain so the tensor queue
            # always has PV work to cover the exp drain latency of scp
            # emit PV chains (and phase-3 chains) in PAIRS: each pv->scores
            # transition costs ~160ns on the first score matmul (LDW not
            # hidden behind the short N=129 PV matmuls), so halve the number
            # of transitions
            si = pi = ti3 = 0
            while si < len(sg) or pi < npv or ti3 < len(p3_nbs):
                for _ in range(4):
                    if si < len(sg):
                        emit_sg(sg[si]); si += 1
                for _ in range(2):
                    if pi < npv:
                        emit_pv(pv_h, pv_qt, pi); pi += 1
                    elif ti3 < len(p3_nbs):
                        pool = opp2 if (opp2 is not None and ti3 % 2) else opp
                        emit_p3(p3_nbs[ti3], pool); ti3 += 1

            # free consumed probs tiles
            if pv_h is not None:
                pq_by.pop((pv_h, pv_qt), None)

        for stk in (pB3, pB2):
            if stk is not None:
                stk.close()

    nc.compile()
    return nc


def _get_nc(n=N_CTX):
    if n not in _CACHE:
        _CACHE[n] = _build(n)
    return _CACHE[n]


def _shard_inputs(q, k, v, Wq, bq, Wk, bk, Wv, bv, Wo, bo):
    bf = ml_dtypes.bfloat16
    in_maps = []
    for c in range(8):
        bi, hg = c // 2, c % 2
        s = slice(hg * CH, (hg + 1) * CH)
        bqk_c = np.empty((128, 2 * NH), np.float32)
        for h in range(NH):
            bqk_c[:, h] = bq[hg * CH + h * HD : hg * CH + (h + 1) * HD]
            bqk_c[:, NH + h] = bk[hg * CH + h * HD : hg * CH + (h + 1) * HD]
        bvb_c = np.ascontiguousarray(
            np.broadcast_to(bv[s].astype(np.float32), (128, CH))
        )
        f8 = ml_dtypes.float8_e4m3fn
        in_maps.append({
            "qT": np.ascontiguousarray(q[bi].T).astype(f8),
            "kT": np.ascontiguousarray(k[bi].T).astype(f8),
            "vT": np.ascontiguousarray(v[bi].T).astype(bf),
            "wq": np.ascontiguousarray(Wq[:, s]).astype(f8),
            "wk": np.ascontiguousarray(Wk[:, s]).astype(f8),
            "wv": np.ascontiguousarray(Wv[:, s]).astype(bf),
            "wo": np.ascontiguousarray(Wo[s, :]).astype(bf),
            "bqk": bqk_c,
            "bvb": bvb_c,
        })
    return in_maps


def kernel(q, k, v, Wq, bq, Wk, bk, Wv, bv, Wo, bo, _profile=False):
    import os

    q = np.asarray(q); k = np.asarray(k); v = np.asarray(v)
    n = q.shape[1]
    nc = _get_nc(n)
    in_maps = _shard_inputs(
        q, k, v, np.asarray(Wq), np.asarray(bq), np.asarray(Wk), np.asarray(bk),
        np.asarray(Wv), np.asarray(bv), np.asarray(Wo), np.asarray(bo),
    )
    profile = _profile or bool(int(os.environ.get("KERNEL_PROFILE", "0")))
    if profile:
        _install_profhook()
    res = run_bass_kernel_spmd(nc, in_maps, list(range(8)), trace=profile)
    if profile and res.exec_time_ns is not None:
        print(f"HW exec time: {res.exec_time_ns} ns")
    bo32 = np.asarray(bo, np.float32)
    out = np.empty((q.shape[0], n, DM), np.float32)
    for bi in range(q.shape[0]):
        out[bi] = res.results[2 * bi]["out"] + res.results[2 * bi + 1]["out"] + bo32
    return out


# revision 21
# speedup vs baseline: 1.2705x; 1.2705x over previous
"""Multi-head attention (b=4, n=4096, d_model=768, 16 heads x 128) on 8 TRN2
NeuronCores.

Sharding: core c handles batch c//2, head-group c%2 (8 heads = 1024 channels).
Host transposes q/k/v per batch to (768, n) and slices weights per head-group;
each core computes its partial output projection (n, 768); host sums the two
head-group partials per batch and adds the output bias.

Per-core dataflow (layouts chosen so no probability transpose is needed):
  phase 1a: qhT/khT = per-head projections producing (head_dim, n) via
           out = lhsT.T @ rhs with lhsT=W chunk, rhs=xT chunk; q/k bias added
           on ScalarE (Identity activation with per-partition bias AP).
  phase 1b: vh in natural (n, ch) layout, NO bias (v-bias folded in post-PV:
           out = pv/denom + bv since sum(probs)=denom).
  phase 2: flat software pipeline over (head, qt) steps that crosses head
           boundaries: each step emits score matmuls + exp for (h, qt)
           interleaved with the PV chains of the previous step's tile, so the
           PE never drains at head starts (scores alone are exp-throughput
           gated; PV work covers the drain).  scoresT (keys, q) =
           khT_chunk.T @ qhT -> probsT via exp split across ScalarE (exact
           Exp) and VectorE (Schraudolph fast-exp) in a 5:4 ratio.  PV per
           q-block: out(q,129) = probsT_chunk.T @ [vh | 1] accumulating the
           softmax denominator in column 128; normalize on VectorE fused with
           the v-bias add, DMA-xbar transpose into attnT (ch, n).
  phase 3: out(n,768) = attnT_chunk.T @ Wo chunks, fp32 out.  Interleaved
           into the last head's pipeline steps (nb-groups emitted two steps
           behind the PV that completes them), with a smaller score PSUM pool
           for that head so PSUM fits scores+PV+out chains simultaneously.
"""

import numpy as np
import ml_dtypes
from contextlib import ExitStack

import concourse.bass as bass
import concourse.mybir as mybir
import concourse.tile as tile
from concourse import bacc
from concourse.bass_utils import run_bass_kernel_spmd

B = 4
N_CTX = 4096
DM = 768
CH = 1024          # channels per core (8 heads x 128)
HD = 128
NH = 8             # heads per core
KC = DM // 128     # 6 contraction chunks for projections
NT = 512           # n-tile width
SCALE = HD ** -0.5
LOG2E = 1.4426950408889634
C_ZM = 7.219274    # zero-mean Schraudolph constant (2^7 mantissa domain)

f32 = mybir.dt.float32
bf16 = mybir.dt.bfloat16
i16 = mybir.dt.int16
fp8 = mybir.dt.float8e4
AF = mybir.ActivationFunctionType
ALU = mybir.AluOpType
PM = mybir.MatmulPerfMode
NTP = 1024         # phase-1 n-tile width (wide moving to amortize LDWEIGHTS)

_CACHE = {}


def _install_profhook():
    import contextlib, ctypes, sys, types

    if "antenv.axon_hooks" in sys.modules:
        return
    so = "/opt/axon/libaxon_pjrt.so"
    try:
        lib = ctypes.CDLL(so)
    except OSError:
        return
    if not hasattr(lib, "axon_start_nrt_profile"):
        return
    lib.axon_start_nrt_profile.argtypes = [ctypes.POINTER(ctypes.c_int64), ctypes.c_size_t]
    lib.axon_start_nrt_profile.restype = ctypes.c_int64
    lib.axon_stop_nrt_profile.argtypes = [ctypes.c_char_p]
    lib.axon_stop_nrt_profile.restype = ctypes.c_int64

    @contextlib.contextmanager
    def _hook(output_dir, device_ids):
        import jax
        jax.devices()
        if device_ids:
            ids = (ctypes.c_int64 * len(device_ids))(*device_ids)
            rc = lib.axon_start_nrt_profile(ids, len(device_ids))
        else:
            rc = lib.axon_start_nrt_profile(None, 0)
        if rc != 0:
            raise RuntimeError(f"axon_start_nrt_profile rc={rc}")
        try:
            yield
        finally:
            nf = lib.axon_stop_nrt_profile(str(output_dir).encode())
            print(f"profile: {nf} ntff file(s) in {output_dir}", file=sys.stderr)

    mod = types.ModuleType("antenv.axon_hooks")
    mod.get_axon_ntff_profile_hook = lambda: _hook
    mod.set_axon_ntff_profile_hook = lambda h: None
    sys.modules["antenv.axon_hooks"] = mod

    import concourse.bass_utils as bu
    bu.upload_artifacts = lambda tmpdir: "local://" + str(tmpdir)


def _build(n=N_CTX):
    nts = n // NT          # n tiles (8)
    nbs = n // 128         # n blocks (32)
    nc = bacc.Bacc(None, target_bir_lowering=False, debug=False, num_devices=8)

    qT = nc.declare_dram_parameter("qT", [DM, n], fp8, isOutput=False)
    kT = nc.declare_dram_parameter("kT", [DM, n], fp8, isOutput=False)
    vT = nc.declare_dram_parameter("vT", [DM, n], bf16, isOutput=False)
    wq = nc.declare_dram_parameter("wq", [DM, CH], fp8, isOutput=False)
    wk = nc.declare_dram_parameter("wk", [DM, CH], fp8, isOutput=False)
    wv = nc.declare_dram_parameter("wv", [DM, CH], bf16, isOutput=False)
    wo = nc.declare_dram_parameter("wo", [CH, DM], bf16, isOutput=False)
    bqk = nc.declare_dram_parameter("bqk", [128, 2 * NH], f32, isOutput=False)
    bvb = nc.declare_dram_parameter("bvb", [128, CH], f32, isOutput=False)
    out = nc.declare_dram_parameter("out", [n, DM], f32, isOutput=True)

    with tile.TileContext(nc) as tc, ExitStack() as ctx:
        dram = ctx.enter_context(tc.tile_pool(name="dram", bufs=1, space="DRAM"))
        qh_s = dram.tile([NH, 128, n], bf16)
        kh_s = dram.tile([NH, 128, n], bf16)
        vh_s = dram.tile([nbs, 128, NH, HD], bf16)

        singles = ctx.enter_context(tc.tile_pool(name="singles", bufs=1))

        bqk_t = singles.tile([128, 2 * NH], f32)
        nc.sync.dma_start(out=bqk_t[:, :], in_=bqk[:, :])
        bvb_t = singles.tile([128, CH], f32)
        nc.sync.dma_start(out=bvb_t[:, :], in_=bvb[:, :])
        # warm the ScalarE exp table set while phase 1 hasn't touched ACT yet
        # (the first real Exp otherwise pays the ~2.7us ACT_TABLE_LOAD right
        # at the phase-1 -> phase-2 transition, stalling the score pipeline)
        scratch = singles.tile([128, 1], f32)
        nc.scalar.activation(scratch[:, :], bqk_t[:, 0:1], AF.Exp)

        # phase-2 pools that phase 1 needs (head-0 score overlap): qh/kh tiles
        # and the probs pool must exist before the last v-projection tile
        GG = 16  # key-chunks per buffered probs tile (2 tiles per qt)
        EG = 3   # key-chunks per exp op (3 psum banks per sc tile)
        sch_s1 = SCALE * LOG2E * 128.0
        sch_s2 = 127.0 * 128.0 - C_ZM
        qk2 = ctx.enter_context(tc.tile_pool(name="qk2", bufs=2))
        probs = ctx.enter_context(tc.tile_pool(name="probs", bufs=4))
        qk_ts = {}
        pq_by = {}    # (h, qt) -> list of probs tiles

        def load_qk(h):
            qh_t = qk2.tile([128, n], bf16, tag="qh", name=f"qh{h}")
            kh_t = qk2.tile([128, n], bf16, tag="kh", name=f"kh{h}")
            for chk in range(n // NTP):
                cs = slice(chk * NTP, (chk + 1) * NTP)
                nc.sync.dma_start(out=qh_t[:, cs], in_=qh_s[h, :, cs])
                nc.sync.dma_start(out=kh_t[:, cs], in_=kh_s[h, :, cs])
            return qh_t, kh_t

        # ---------------- phase 1a: q/k projections ----------------
        p1 = ExitStack()
        wpool = p1.enter_context(tc.tile_pool(name="w1", bufs=1))
        stream = p1.enter_context(tc.tile_pool(name="stream", bufs=3))
        stage1 = p1.enter_context(tc.tile_pool(name="stage1", bufs=4))
        pp = p1.enter_context(tc.tile_pool(name="pp", bufs=4, space="PSUM"))

        wq_t = wpool.tile([128, KC, CH], fp8, tag="wq")
        wk_t = wpool.tile([128, KC, CH], fp8, tag="wk")
        wv_t = wpool.tile([128, KC, CH], bf16, tag="wv")
        for w_t, w in ((wq_t, wq), (wk_t, wk), (wv_t, wv)):
            nc.sync.dma_start(
                out=w_t[:, :, :], in_=w[:].rearrange("(c p) m -> p c m", p=128)
            )

        ntsp = n // NTP
        for nt in range(ntsp):
            xs = []
            for name, x in (("q", qT), ("k", kT)):
                x_t = stream.tile([128, KC, NTP], fp8, tag=f"s{name}")
                # per-chunk-pair DMAs: the first DR matmul needs only pair 0,
                # so projections start before the whole tile lands
                for cp_ in range(KC // 2):
                    nc.sync.dma_start(
                        out=x_t[:, 2 * cp_ : 2 * cp_ + 2, :],
                        in_=x[:].rearrange("(c p) m -> p c m", p=128)[
                            :, 2 * cp_ : 2 * cp_ + 2, nt * NTP : (nt + 1) * NTP
                        ],
                    )
                xs.append(x_t)
            xv_t = stream.tile([128, KC, NTP], bf16, tag="sv", bufs=2)
            nc.sync.dma_start(
                out=xv_t[:, :, :],
                in_=vT[:].rearrange("(c p) m -> p c m", p=128)[
                    :, :, nt * NTP : (nt + 1) * NTP
                ],
            )
            for ti, (x_t, sc) in enumerate(((xs[0], qh_s), (xs[1], kh_s))):
                w_t = (wq_t, wk_t)[ti]
                for h in range(NH):
                    ps = pp.tile([128, NTP], f32, tag="proj")
                    # fp8 DoubleRow: contract chunk pair (2c, 2c+1); fp8
                    # moving operand caps at 1024 elems so go per 512-half
                    for half in range(NTP // 512):
                        hs = slice(half * 512, (half + 1) * 512)
                        for c in range(KC // 2):
                            nc.tensor.matmul(
                                ps[:, hs],
                                w_t[:, 2 * c : 2 * c + 2, h * HD : (h + 1) * HD],
                                x_t[:, 2 * c : 2 * c + 2, hs],
                                start=(c == 0),
                                stop=(c == KC // 2 - 1),
                                perf_mode=PM.DoubleRow,
                                skip_group_check=True,
                            )
                    st = stage1.tile([128, NTP], bf16, tag="qkst")
                    # bias add on ScalarE (idle during phase 1)
                    nc.scalar.activation(
                        st[:, :], ps[:, :], AF.Identity,
                        bias=bqk_t[:, ti * NH + h : ti * NH + h + 1],
                    )
                    nc.sync.dma_start(
                        out=sc[h, :, nt * NTP : (nt + 1) * NTP], in_=st[:, :]
                    )
            # v projection for this n-tile (no bias; folded post-PV)
            def emit_vproj(nt_, nbq, xv):
                ps = pp.tile([128, CH], f32, tag="proj", name="psv")
                for ct in range(CH // 512):
                    cs = slice(ct * 512, (ct + 1) * 512)
                    for c in range(KC):
                        nc.tensor.matmul(
                            ps[:, cs],
                            xv[:, c, nbq * 128 : (nbq + 1) * 128],
                            wv_t[:, c, cs],
                            start=(c == 0),
                            stop=(c == KC - 1),
                            skip_group_check=True,
                        )
                st = stage1.tile([128, CH], bf16, tag="vst")
                nc.vector.tensor_copy(st[:, :], ps[:, :])
                nc.sync.dma_start(
                    out=vh_s[nt_ * (NTP // 128) + nbq, :, :, :], in_=st[:, :],
                )

            if nt < ntsp - 1:
                for nbq in range(NTP // 128):
                    emit_vproj(nt, nbq, xv_t)
            else:
                # last n-tile: interleave head-0 qt-0 scores with the
                # v-projection so the transition into phase 2 never drains the
                # PE (score groups alone are exp-throughput gated). Score psums
                # borrow the phase-1 "proj" pool slots (eg=2 -> same 4KB size).
                qk_ts[0] = load_qk(0)
                pq00 = []
                for gq in range(2):
                    pq0 = probs.tile([128, GG, NT], bf16, tag="pq",
                                     name=f"pq0_0_{gq}")
                    pq00.append(pq0)
                sg00 = [(pq00[g // 8], (g // 8) * GG, (g % 8) * 2)
                        for g in range(16)]
                gi = 0
                sched = {2: 2, 3: 3, 4: 3, 5: 3, 6: 3, 7: 2}

                def emit_sg00(task):
                    pq0, kb0, off = task
                    sc_t = pp.tile([128, 2, NT], f32, tag="proj", name="sc00")
                    for i in range(2):
                        kb = kb0 + off + i
                        nc.tensor.matmul(
                            sc_t[:, i, :],
                            qk_ts[0][1][:, kb * 128 : (kb + 1) * 128],
                            qk_ts[0][0][:, 0:NT],
                            start=True, stop=True,
                        )
                    # 2 ScalarE : 1 VectorE here — the DVE carries the
                    # v-projection psum drains in this window
                    if gi % 3 < 2:
                        nc.scalar.activation(
                            pq0[:, off : off + 2, :], sc_t[:, :, :],
                            AF.Exp, scale=SCALE,
                        )
                    else:
                        nc.vector.tensor_scalar(
                            pq0[:, off : off + 2, :].bitcast(i16),
                            sc_t[:, :, :],
                            sch_s1, sch_s2, ALU.mult, ALU.add,
                        )

                for nbq in range(NTP // 128):
                    emit_vproj(nt, nbq, xv_t)
                    for _ in range(sched.get(nbq, 0)):
                        emit_sg00(sg00[gi]); gi += 1
                while gi < 16:
                    emit_sg00(sg00[gi]); gi += 1
                pq_by[(0, 0)] = pq00

        p1.close()

        # ---------------- phase 2+3: flat pipelined attention ----------------
        atile = ctx.enter_context(tc.tile_pool(name="atile", bufs=NH))
        vh2 = ctx.enter_context(tc.tile_pool(name="vh2", bufs=2))
        stage2 = ctx.enter_context(tc.tile_pool(name="stage2", bufs=4))
        small = ctx.enter_context(tc.tile_pool(name="small", bufs=4))
        stage3 = ctx.enter_context(tc.tile_pool(name="stage3", bufs=2))
        wpool3 = ctx.enter_context(tc.tile_pool(name="w3", bufs=1))
        pvp = ctx.enter_context(tc.tile_pool(name="pvp", bufs=2, space="PSUM"))

        # output-projection weights: load early, plenty of DMA slack
        wo_t = wpool3.tile([128, NH, DM], bf16, tag="wo")
        nc.sync.dma_start(
            out=wo_t[:, :, :], in_=wo[:].rearrange("(c p) m -> p c m", p=128)
        )

        # scores psum: big pool for heads 0..NH-2, small for the last head so
        # the interleaved phase-3 chains get PSUM banks
        pA = ExitStack()
        scpA = pA.enter_context(tc.tile_pool(name="scpA", bufs=2, space="PSUM"))
        pB1 = pB2 = pB3 = None
        scpB = None
        opp = None
        opp2 = None

        def load_vh(h):
            vh_t = vh2.tile([128, nbs, HD + 1], bf16, tag="vh", name=f"vh{h}")
            for chk in range(4):
                bs = slice(chk * (nbs // 4), (chk + 1) * (nbs // 4))
                nc.sync.dma_start(
                    out=vh_t[:, bs, 0:HD],
                    in_=vh_s[bs, :, h, :].rearrange("c p j -> p c j"),
                )
            nc.vector.memset(vh_t[:, :, HD], 1.0)
            return vh_t

        vh_ts = {0: load_vh(0)}
        at_ts = []
        expctr = 1    # global exp-group counter; strict scalar/vector
        # alternation (a 5:4 pattern stalls the score psum rotation at its
        # wrap where two consecutive groups land on ScalarE). Starts at 1:
        # measured marginally better rel-err than the even-parity start, same
        # speed.

        def emit_sg(task):
            nonlocal expctr
            scp, eg, pq, kb0, off, gsz, h, qt = task
            kh_t = qk_ts[h][1]
            qh_t = qk_ts[h][0]
            sc_t = scp.tile([128, eg, NT], f32, tag="sc")
            for i in range(gsz):
                kb = kb0 + off + i
                nc.tensor.matmul(
                    sc_t[:, i, :],
                    kh_t[:, kb * 128 : (kb + 1) * 128],
                    qh_t[:, qt * NT : (qt + 1) * NT],
                    start=True,
                    stop=True,
                )
            if expctr % 2 == 0:
                nc.scalar.activation(
                    pq[:, off : off + gsz, :], sc_t[:, 0:gsz, :],
                    AF.Exp, scale=SCALE,
                )
            else:
                nc.vector.tensor_scalar(
                    pq[:, off : off + gsz, :].bitcast(i16),
                    sc_t[:, 0:gsz, :],
                    sch_s1, sch_s2, ALU.mult, ALU.add,
                )
            expctr += 1

        def emit_pv(h, qt, qb):
            plist = pq_by[(h, qt)]
            vh_t = vh_ts[h]
            at_t = at_ts[h]
            pv = pvp.tile([128, HD + 1], f32, tag="pv")
            for kb in range(nbs):
                nc.tensor.matmul(
                    pv[:, :],
                    plist[kb // GG][:, kb % GG, qb * 128 : (qb + 1) * 128],
                    vh_t[:, kb, :],
                    start=(kb == 0),
                    stop=(kb == nbs - 1),
                    skip_group_check=True,
                )
            rec = small.tile([128, 1], f32, tag="rec")
            nc.vector.reciprocal(rec[:, :], pv[:, HD : HD + 1])
            st = stage2.tile([128, HD], bf16, tag="nst")
            # st = pv/denom + bv  (one fused DVE op)
            nc.vector.scalar_tensor_tensor(
                st[:, :], pv[:, 0:HD], rec[:, :],
                bvb_t[:, h * HD : (h + 1) * HD],
                ALU.mult, ALU.add,
            )
            qb_g = qt * 4 + qb
            # transpose via DMA xbar (frees TensorE + avoids queue stall)
            nc.sync.dma_start_transpose(
                out=at_t[:, qb_g * 128 : (qb_g + 1) * 128], in_=st[:, :]
            )

        def emit_p3(nb, pool=None):
            po = (pool or opp).tile([128, DM], f32, tag="po")
            for hh in range(NH):
                lhsT = at_ts[hh][:, nb * 128 : (nb + 1) * 128]
                nc.tensor.matmul(
                    po[:, 0:512], lhsT, wo_t[:, hh, 0:512],
                    start=(hh == 0), stop=(hh == NH - 1), skip_group_check=True,
                )
                nc.tensor.matmul(
                    po[:, 512:DM], lhsT, wo_t[:, hh, 512:DM],
                    start=(hh == 0), stop=(hh == NH - 1), skip_group_check=True,
                )
            so = stage3.tile([128, DM], f32, tag="ost")
            # drain on ScalarE: the last head's DVE is busy with exp + PV
            # normalization; ACT has slack there
            nc.scalar.activation(so[:, :], po[:, :], AF.Identity)
            nc.sync.dma_start(out=out[nb * 128 : (nb + 1) * 128, :], in_=so[:, :])

        # step list: (h, qt) with qt < nts emits scores for (h, qt); every step
        # emits PV for the previous (h, qt) pair; last head gets a sentinel
        # step and interleaved phase-3 groups (two steps behind).
        steps = [(h, qt) for h in range(NH) for qt in range(nts)]
        steps.append((NH - 1, nts))  # sentinel: final PV + phase-3 tail

        for h, qt in steps:
            last_head = h == NH - 1
            if qt == 0:
                at_t = atile.tile([128, n], bf16, tag="at", name=f"at{h}")
                at_ts.append(at_t)
                if h + 1 < NH:
                    qk_ts[h + 1] = load_qk(h + 1)
                if h == NH - 1:
                    # switch to the narrower scores pool (EG=2, still double
                    # buffered) + open phase-3 psum
                    pA.close()
                    pB2 = ExitStack()
                    opp = pB2.enter_context(
                        tc.tile_pool(name="opp", bufs=1, space="PSUM"))
                    pB1 = ExitStack()
                    scpB = pB1.enter_context(
                        tc.tile_pool(name="scpB", bufs=2, space="PSUM"))
            if qt == 1 and h + 1 < NH:
                vh_ts[h + 1] = load_vh(h + 1)

            scp = scpB if last_head else scpA
            eg = 2 if last_head else EG

            # score-group tasks for this (h, qt); (0,0) was emitted inside
            # phase 1, interleaved with the last v-projection tile
            sg = []
            if qt < nts and not (h == 0 and qt == 0):
                pq_list = []
                for gq in range(nbs // GG):
                    kb0 = gq * GG
                    pq = probs.tile([128, GG, NT], bf16, tag="pq",
                                    name=f"pq{h}_{qt}_{gq}")
                    pq_list.append(pq)
                    off = 0
                    while off < GG:
                        gsz = min(eg, GG - off)
                        sg.append((scp, eg, pq, kb0, off, gsz, h, qt))
                        off += gsz
                pq_by[(h, qt)] = pq_list

            # PV chains for the previous step's scores
            if qt > 0:
                pv_h, pv_qt = h, qt - 1
            elif h > 0:
                pv_h, pv_qt = h - 1, nts - 1
            else:
                pv_h = None
            npv = 4 if pv_h is not None else 0

            # phase-3 groups: two steps behind the PV completion on last head
            p3_nbs = []
            if last_head and qt >= 2:
                blk = qt - 2
                p3_nbs = list(range(blk * 4, blk * 4 + 4))
            if last_head and qt == nts:
                p3_nbs += list(range((nts - 1) * 4, (nts - 1) * 4 + 4))
                # scores are done: free the score psum and open a second
                # phase-3 psum so the tail chains double-buffer their drains
                pB1.close()
                pB3 = ExitStack()
                opp2 = pB3.enter_context(
                    tc.tile_pool(name="opp2", bufs=1, space="PSUM"))

            # interleave: 2 score groups per PV chain so the tensor queue
            # always has PV work to cover the exp drain latency of scp
            # emit PV chains (and phase-3 chains) in PAIRS: each pv->scores
            # transition costs ~160ns on the first score matmul (LDW not
            # hidden behind the short N=129 PV matmuls), so halve the number
            # of transitions
            si = pi = ti3 = 0
            while si < len(sg) or pi < npv or ti3 < len(p3_nbs):
                for _ in range(4):
                    if si < len(sg):
                        emit_sg(sg[si]); si += 1
                for _ in range(2):
                    if pi < npv:
                        emit_pv(pv_h, pv_qt, pi); pi += 1
                    elif ti3 < len(p3_nbs):
                        pool = opp2 if (opp2 is not None and ti3 % 2) else opp
                        emit_p3(p3_nbs[ti3], pool); ti3 += 1

            # free consumed probs tiles
            if pv_h is not None:
                pq_by.pop((pv_h, pv_qt), None)

        for stk in (pB3, pB2):
            if stk is not None:
                stk.close()

    nc.compile()
    return nc


def _get_nc(n=N_CTX):
    if n not in _CACHE:
        _CACHE[n] = _build(n)
    return _CACHE[n]


def _shard_inputs(q, k, v, Wq, bq, Wk, bk, Wv, bv, Wo, bo):
    bf = ml_dtypes.bfloat16
    in_maps = []
    for c in range(8):
        bi, hg = c // 2, c % 2
        s = slice(hg * CH, (hg + 1) * CH)
        bqk_c = np.empty((128, 2 * NH), np.float32)
        for h in range(NH):
            bqk_c[:, h] = bq[hg * CH + h * HD : hg * CH + (h + 1) * HD]
            bqk_c[:, NH + h] = bk[hg * CH + h * HD : hg * CH + (h + 1) * HD]
        bvb_c = np.ascontiguousarray(
            np.broadcast_to(bv[s].astype(np.float32), (128, CH))
        )
        f8 = ml_dtypes.float8_e4m3fn
        in_maps.append({
            "qT": np.ascontiguousarray(q[bi].T).astype(f8),
            "kT": np.ascontiguousarray(k[bi].T).astype(f8),
            "vT": np.ascontiguousarray(v[bi].T).astype(bf),
            "wq": np.ascontiguousarray(Wq[:, s]).astype(f8),
            "wk": np.ascontiguousarray(Wk[:, s]).astype(f8),
            "wv": np.ascontiguousarray(Wv[:, s]).astype(bf),
            "wo": np.ascontiguousarray(Wo[s, :]).astype(bf),
            "bqk": bqk_c,
            "bvb": bvb_c,
        })
    return in_maps


def kernel(q, k, v, Wq, bq, Wk, bk, Wv, bv, Wo, bo, _profile=False):
    import os

    q = np.asarray(q); k = np.asarray(k); v = np.asarray(v)
    n = q.shape[1]
    nc = _get_nc(n)
    in_maps = _shard_inputs(
        q, k, v, np.asarray(Wq), np.asarray(bq), np.asarray(Wk), np.asarray(bk),
        np.asarray(Wv), np.asarray(bv), np.asarray(Wo), np.asarray(bo),
    )
    profile = _profile or bool(int(os.environ.get("KERNEL_PROFILE", "0")))
    if profile:
        _install_profhook()
    res = run_bass_kernel_spmd(nc, in_maps, list(range(8)), trace=profile)
    if profile and res.exec_time_ns is not None:
        print(f"HW exec time: {res.exec_time_ns} ns")
    bo32 = np.asarray(bo, np.float32)
    out = np.empty((q.shape[0], n, DM), np.float32)
    for bi in range(q.shape[0]):
        out[bi] = res.results[2 * bi]["out"] + res.results[2 * bi + 1]["out"] + bo32
    return out
